# revision 16
# baseline (speedup 1.0000x reference)
"""Trainium2 Bass kernel for nn_MinimalGazeEncoder.

Data-parallel over batch: 8 cores x 8 batch elements each.

Per-core layout: partition p = b*16 + c over 128 chunks of 512 timesteps
(b in [0,8), c in [0,16)).  P[128, 21*512] (bf16) holds one [128, 512]
plane per feature channel in reference order (slot 20 = ones for the b1
row); SCR[128, 15*512] (f32) holds full-precision intermediates, so every
feature is computed in f32 and rounded to bf16 exactly once.

Time-shift chunk boundaries (causal diff) and the EMA chunk carries use a
small constant shift matrix on the PE (shift by one chunk within each
batch element); matmul rhs reads the boundary columns via strided views
and the carry consumers read PSUM directly (no staging copies).  The EMA
is a hardware prefix scan (tensor_tensor_scan) plus a rank-1 alpha-powers
carry fixup (alpha^512 underflows so carries never chain).

Matmuls: G-tiles [128, 512] (bf16) gather the 21 feature rows of 4 chunks
onto partitions 32g+s with one SWDGE DMA per tile; layer 1 runs as 4
concurrent row-tiled K=21 bf16 matmuls (W1|b1 replicated at partitions
0/32/64/96).  Layer 2 is flipped (lhsT = strided h1 column views, rhs =
W2): matmul l uses h1 columns {l, 16+l, ..., 2032+l} so the output lands
as [t_block, (l, d)] with t = 16*t_block + l -- each PSUM partition then
maps to a CONTIGUOUS 16-timestep x 128-d run of the HBM output, giving
8 KB DMA descriptors instead of 512 B ones.  The b2 PSUM pre-fill matmuls
are skipped when b2 == 0 (setup_inputs produces zeros); a fill path is
kept for nonzero b2.  The per-tile loop is software-pipelined (L1+gelu1
of tile i runs before L2+gelu2 of tile i-1) so the ACT engine never waits
on the PE round trip with single-buffered PSUM.
"""

import math

import numpy as np
import ml_dtypes

import concourse.bacc as bacc
import concourse.tile as tile
import concourse.mybir as mybir
from concourse.bass_utils import run_bass_kernel_spmd

F32 = mybir.dt.float32
F32R = mybir.dt.float32r
BF16 = mybir.dt.bfloat16
AF = mybir.ActivationFunctionType
ALU = mybir.AluOpType

B, T, D_OUT = 64, 8192, 128
KPOS = 2
DT = 1.0 / 240.0
N_CORES = 8
BL = B // N_CORES          # 8 batch elements per core
CH = 512                   # timesteps per chunk
CPB = T // CH              # 16 chunks per batch element
NP = BL * CPB              # 128 chunks = partitions
GT = 4                     # chunks per G-tile
NGT = NP // GT             # 32 G-tiles per core
SLOTS = 32                 # feature-slot stride in P
NSLOT = 21                 # feature slots (20 features + ones row)

L2_DT = BF16               # layer-2 lhsT/rhs dtype
L1_DT = BF16               # layer-1 lhsT/rhs dtype (G tiles, W1b)

ALPHA_F, ALPHA_S = 0.8, 0.95

# P slot indices (bf16 feature planes, reference feature order)
S_FX = 0         # 0..3  sin(x,k0) sin(x,k1) cos(x,k0) cos(x,k1)
S_FY = 4         # 4..7
S_VX, S_VY, S_SPD, S_DC, S_DS = 8, 9, 10, 11, 12
S_AX, S_AY, S_APAR, S_APERP = 13, 14, 15, 16
S_GATE, S_QF, S_QS = 17, 18, 19
S_ONES = 20

# SCR slot indices (f32 intermediates). XD/YD, VDX/VDY, QF/QS pairs must
# stay adjacent: the carry matmuls read their boundary columns as one
# strided 2-column rhs.  QF/QS reuse the gaze staging slots (stage is
# fully consumed by the XD/YD ops long before the scans run).
C_XD, C_YD, C_VX, C_VY, C_VDX, C_VDY = 0, 1, 2, 3, 4, 5
C_AX, C_AY, C_SPD, C_ISP, C_TA, C_TB, C_TC = 6, 7, 8, 9, 10, 11, 12
C_QF, C_QS, C_TB2, C_TD, C_TE = 13, 14, 15, 16, 17
C_STAGE = 13     # 13..14: raw interleaved gaze staging [128, 1024]

_cache = {}


def _np_dt(dt):
    return np.float32 if dt in (F32, F32R) else ml_dtypes.bfloat16


def _build_nc(use_b2, fcoef, fphi, inv_t, nthr):
    nc = bacc.Bacc("TRN2", target_bir_lowering=False, debug=False,
                   num_devices=N_CORES)

    d_gaze = nc.dram_tensor("gaze", [BL, T, 2], F32, kind="ExternalInput")
    d_W1b = nc.dram_tensor("W1b", [128, 128], L1_DT, kind="ExternalInput")
    d_W2 = nc.dram_tensor("W2", [128, 128], L2_DT, kind="ExternalInput")
    if use_b2:
        d_ones1 = nc.dram_tensor("ones1", [1, 128], L2_DT,
                                 kind="ExternalInput")
        d_b2rep = nc.dram_tensor("b2rep", [1, CH], L2_DT,
                                 kind="ExternalInput")
    d_S = nc.dram_tensor("Smat", [128, 128], F32, kind="ExternalInput")
    d_APOW = nc.dram_tensor("APOW", [128, 2 * CH], F32, kind="ExternalInput")
    d_SCAL = nc.dram_tensor("SCAL", [128, 16], F32, kind="ExternalInput")
    d_out = nc.dram_tensor("out", [BL, T, D_OUT], BF16,
                              kind="ExternalOutput")

    PI = float(np.pi)

    with tile.TileContext(nc) as tc:
        with (
            tc.tile_pool(name="pP", bufs=1) as pP,
            tc.tile_pool(name="pC", bufs=1) as pC,
            tc.tile_pool(name="pG", bufs=5) as pG,
            tc.tile_pool(name="pH", bufs=3) as pH,
            tc.tile_pool(name="pO", bufs=5) as pO,
            tc.tile_pool(name="ps1", bufs=1, space="PSUM") as ps1,
            tc.tile_pool(name="ps2", bufs=2, space="PSUM") as ps2,
        ):
            P = pP.tile([128, SLOTS * CH], BF16)
            SCR = pP.tile([128, 18 * CH], F32)

            def sl(i, n=1):
                return P[:, i * CH:(i + n) * CH]

            def sc(i, n=1):
                return SCR[:, i * CH:(i + n) * CH]

            # gaze staging DMA first: everything in phase A waits on it,
            # so it must head the sync queue.
            stage = sc(C_STAGE, 2)
            gz = d_gaze[:].rearrange("b t two -> b (t two)") \
                          .rearrange("b (c f) -> (b c) f", f=2 * CH)
            nc.sync.dma_start(out=stage[:, 0:CH], in_=gz[:, 0:CH])
            nc.scalar.dma_start(out=stage[:, CH:2 * CH], in_=gz[:, CH:2 * CH])

            # constants / weights on the (idle at startup) ACT queue, in
            # order of first use; the big APOW plane goes last.
            t_SCAL = pC.tile([128, 16], F32, tag="SCAL")
            nc.scalar.dma_start(out=t_SCAL[:], in_=d_SCAL[:])
            t_S = pC.tile([128, 128], F32, tag="Smat")
            nc.scalar.dma_start(out=t_S[:], in_=d_S[:])

            # Preload the trig ACT table (one set is resident at a time;
            # the fourier sins run first) with a dummy [128,1] op while
            # the gaze staging DMA is in flight.
            t_dm = pC.tile([128, 1], F32, tag="tdm")
            nc.gpsimd.memset(t_dm[:], 1.0)
            t_dmo = pC.tile([128, 1], F32, tag="tdmo")
            nc.scalar.activation(t_dmo[:], t_dm[:], AF.Sin)

            t_W1b = pC.tile([128, 128], L1_DT, tag="W1b")
            nc.scalar.dma_start(out=t_W1b[:], in_=d_W1b[:])
            t_W2 = pC.tile([128, 128], L2_DT, tag="W2")
            nc.scalar.dma_start(out=t_W2[:], in_=d_W2[:])
            t_APOW = pC.tile([128, 2 * CH], F32, tag="APOW")
            nc.scalar.dma_start(out=t_APOW[:], in_=d_APOW[:])
            if use_b2:
                t_ones1 = pC.tile([1, 128], L2_DT, tag="ones1")
                nc.scalar.dma_start(out=t_ones1[:], in_=d_ones1[:])
                t_b2rep = pC.tile([1, CH], L2_DT, tag="b2rep")
                nc.scalar.dma_start(out=t_b2rep[:], in_=d_b2rep[:])

            # EMA scan multiplier planes: constant per half, memset on the
            # (idle) Pool engine instead of a 512 KB DMA.
            t_ALPH = pC.tile([128, 2 * CH], F32, tag="ALPH")
            nc.gpsimd.memset(t_ALPH[:, 0:CH], ALPHA_F)
            nc.gpsimd.memset(t_ALPH[:, CH:2 * CH], ALPHA_S)
            nc.gpsimd.memset(sl(S_ONES), 1.0)

            # ---- phase A: feature planes ----
            # Scheduled for DVE/ACT overlap: the v/speed chain runs on DVE
            # while ACT does the cross-engine hops (VD copy, sqrt, +eps,
            # sigmoid, scan-input scales) with tables preloaded; the 8
            # fourier sins and the bf16 casts fill the ACT tail while DVE
            # finishes scans+fixups.
            xs = stage.rearrange("p (t two) -> p two t", two=2)
            x_raw, y_raw = xs[:, 0, :], xs[:, 1, :]

            nc.vector.tensor_scalar_mul(sc(C_XD), x_raw, 1.0 / DT)
            nc.vector.tensor_scalar_mul(sc(C_YD), y_raw, 1.0 / DT)

            def last_cols(base):
                # [128, 2] strided view of the boundary columns of the
                # adjacent planes (base, base+1)
                return SCR[:, base * CH:(base + 2) * CH].rearrange(
                    "p (s f) -> p f s", s=2)[:, CH - 1, :]

            # chunk-boundary carries for v
            psA = ps1.tile([128, 2048], F32, tag="ps1")
            nc.tensor.matmul(psA[:, 0:2], t_S[:], last_cols(C_XD),
                             start=True, stop=True)
            for s_v, s_c, col in ((C_VX, C_XD, 0), (C_VY, C_YD, 1)):
                nc.vector.tensor_tensor(
                    sc(s_v)[:, 1:], sc(s_c)[:, 1:], sc(s_c)[:, :-1],
                    ALU.subtract)
                nc.vector.tensor_tensor(
                    sc(s_v)[:, 0:1], sc(s_c)[:, 0:1], psA[:, col:col + 1],
                    ALU.subtract)

            # first chunk of each batch element: v[0] = 0 (prepended frame)
            v0 = SCR[:, C_VX * CH:(C_VX + 2) * CH].rearrange(
                "p (s f) -> p f s", s=2)[:, 0, :]
            nc.vector.tensor_scalar_mul(v0, v0, t_SCAL[:, 10:11])

            # v/dt on ACT (frees DVE for the speed chain)
            nc.scalar.mul(sc(C_VDX, 2), sc(C_VX, 2), 1.0 / DT)

            # fourier sins early on ACT (trig table already resident, and
            # they only need XD/YD): sin(x*s + phi) via fused scale/bias;
            # cos as sin(arg + pi/2) with host-precomputed phases.  Range
            # reduction is skipped: any residual error sits on unit-scale
            # features contributing ~1e-8 of the output L2 norm.
            for ax_i, (s_base, s_src) in enumerate(
                    ((S_FX, C_XD), (S_FY, C_YD))):
                for k in range(KPOS):
                    wc = 2 * ax_i + k
                    nc.scalar.activation(
                        sl(s_base + k), sc(s_src), AF.Sin,
                        bias=t_SCAL[:, 4 + wc:5 + wc], scale=fcoef[wc])
                    nc.scalar.activation(
                        sl(s_base + KPOS + k), sc(s_src), AF.Sin,
                        bias=t_SCAL[:, 11 + wc:12 + wc], scale=fcoef[wc])

            # speed chain on DVE while ACT copies VD
            nc.vector.tensor_tensor(sc(C_TA, 2), sc(C_VX, 2), sc(C_VX, 2),
                                    ALU.mult)
            nc.vector.tensor_tensor(sc(C_TA), sc(C_TA), sc(C_TB), ALU.add)
            nc.scalar.activation(sc(C_SPD), sc(C_TA), AF.Sqrt)
            nc.scalar.add(sc(C_TB), sc(C_SPD), t_SCAL[:, 15:16])
            # gate = sigmoid(invT*speed - invT*thr), affine fused into ACT
            nc.scalar.activation(sc(C_TC), sc(C_SPD), AF.Sigmoid,
                                 bias=t_SCAL[:, 9:10], scale=inv_t)
            # scan inputs (1-alpha)*gate on ACT
            nc.scalar.mul(sc(C_TA), sc(C_TC), 1.0 - ALPHA_F)
            nc.scalar.mul(sc(C_TB2), sc(C_TC), 1.0 - ALPHA_S)

            # a = diff(v/dt) with carries
            psB = ps1.tile([128, 2048], F32, tag="ps1")
            nc.tensor.matmul(psB[:, 0:2], t_S[:], last_cols(C_VDX),
                             start=True, stop=True)
            for s_a, s_c, col in ((C_AX, C_VDX, 0), (C_AY, C_VDY, 1)):
                nc.vector.tensor_tensor(
                    sc(s_a)[:, 1:], sc(s_c)[:, 1:], sc(s_c)[:, :-1],
                    ALU.subtract)
                nc.vector.tensor_tensor(
                    sc(s_a)[:, 0:1], sc(s_c)[:, 0:1], psB[:, col:col + 1],
                    ALU.subtract)

            # 1/(speed+eps), direction, a_par, a_perp on DVE
            nc.vector.reciprocal_approx_accurate(sc(C_ISP), sc(C_TB),
                                                 sc(C_TD))
            nc.vector.tensor_tensor(sl(S_DC), sc(C_VX), sc(C_ISP), ALU.mult)
            nc.vector.tensor_tensor(sl(S_DS), sc(C_VY), sc(C_ISP), ALU.mult)
            nc.vector.tensor_tensor(sc(C_TD, 2), sc(C_VX, 2), sc(C_AX, 2),
                                    ALU.mult)
            nc.vector.tensor_tensor(sc(C_TD), sc(C_TD), sc(C_TE), ALU.add)
            nc.vector.tensor_tensor(sl(S_APAR), sc(C_TD), sc(C_ISP),
                                    ALU.mult)
            nc.vector.tensor_tensor(sc(C_TD), sc(C_VX), sc(C_AY), ALU.mult)
            nc.vector.tensor_tensor(sc(C_TE), sc(C_VY), sc(C_AX), ALU.mult)
            nc.vector.tensor_tensor(sc(C_TD), sc(C_TD), sc(C_TE),
                                    ALU.subtract)
            nc.vector.tensor_tensor(sl(S_APERP), sc(C_TD), sc(C_ISP),
                                    ALU.mult)

            # EMA scans (within-chunk) + carry fixup
            nc.vector.tensor_tensor_scan(
                sc(C_QF), t_ALPH[:, 0:CH], sc(C_TA), 0.0, ALU.mult, ALU.add)
            nc.vector.tensor_tensor_scan(
                sc(C_QS), t_ALPH[:, CH:2 * CH], sc(C_TB2), 0.0,
                ALU.mult, ALU.add)
            psC = ps1.tile([128, 2048], F32, tag="ps1")
            nc.tensor.matmul(psC[:, 0:2], t_S[:], last_cols(C_QF),
                             start=True, stop=True)
            nc.vector.scalar_tensor_tensor(
                sl(S_QF), t_APOW[:, 0:CH], psC[:, 0:1], sc(C_QF),
                ALU.mult, ALU.add)
            nc.vector.scalar_tensor_tensor(
                sl(S_QS), t_APOW[:, CH:2 * CH], psC[:, 1:2], sc(C_QS),
                ALU.mult, ALU.add)

            # ACT tail: bf16 casts (Copy runs under any resident table).
            nc.scalar.copy(sl(S_VX, 2), sc(C_VX, 2))
            nc.scalar.copy(sl(S_AX, 2), sc(C_AX, 2))
            nc.scalar.copy(sl(S_SPD), sc(C_SPD))
            nc.scalar.copy(sl(S_GATE), sc(C_TC))

            # ---- phase B: software-pipelined per-G-tile matmul loop ----
            def g_fetch(i, eng=None):
                G = pG.tile([128, CH], L1_DT, tag="G")
                (eng or nc.gpsimd).dma_start(
                    out=G[:],
                    in_=P[4 * i:4 * i + 4, :].rearrange(
                        "p (s f) -> p s f", s=SLOTS),
                )
                return G

            def l1_front(i, G):
                ps_l1 = ps1.tile([128, 2048], F32, tag="ps1")
                for g in range(GT):
                    nc.tensor.matmul(
                        ps_l1[:, CH * g:CH * (g + 1)],
                        t_W1b[32 * g:32 * g + NSLOT, :],
                        G[32 * g:32 * g + NSLOT, :],
                        start=True, stop=True,
                        tile_position=(32 * g, 0),
                    )
                h1 = pH.tile([128, 2048], L2_DT, tag="h1")
                nc.scalar.activation(h1[:], ps_l1[:], AF.Gelu)
                return h1

            def l2_back(i, h1):
                # two 2-bank PSUM halves (pool bufs=2): L2 of tile i+1 can
                # start on half a while gelu2(i) is still reading half b,
                # breaking the serial gelu2 -> L2 -> gelu2 chain.
                h1v = h1.rearrange("p (c l) -> p l c", l=16)
                o_t = pO.tile([128, 2048], BF16, tag="o")
                b = (4 * i) // CPB
                c0 = (4 * i) % CPB
                dst = d_out[b, c0 * CH:(c0 + 4) * CH, :].rearrange(
                    "(p l) d -> p (l d)", p=128)
                for h in range(2):
                    ps_l2 = ps2.tile([128, 1024], F32, tag="ps2")
                    if use_b2:
                        for jj in range(2):
                            nc.tensor.matmul(
                                ps_l2[:, CH * jj:CH * (jj + 1)],
                                t_ones1[:], t_b2rep[:],
                                start=True, stop=False, skip_group_check=True)
                        for l in range(8):
                            nc.tensor.matmul(
                                ps_l2[:, 128 * l:128 * (l + 1)],
                                h1v[:, 8 * h + l, :], t_W2[:],
                                start=False, stop=True, skip_group_check=True)
                    else:
                        for l in range(8):
                            nc.tensor.matmul(
                                ps_l2[:, 128 * l:128 * (l + 1)],
                                h1v[:, 8 * h + l, :], t_W2[:],
                                start=True, stop=True)
                    nc.scalar.activation(
                        o_t[:, 1024 * h:1024 * (h + 1)], ps_l2[:], AF.Gelu)
                    if i >= NGT - 2:
                        nc.gpsimd.dma_start(
                            out=dst[:, 1024 * h:1024 * (h + 1)],
                            in_=o_t[:, 1024 * h:1024 * (h + 1)])
                if i < NGT - 2:
                    nc.gpsimd.dma_start(out=dst, in_=o_t[:])

            def g_fetch0():
                # first tile: split the gather across three DMA queues so
                # its latency off the end of phase A is minimal
                G = pG.tile([128, CH], L1_DT, tag="G")
                for g, eng in ((0, nc.sync), (1, nc.scalar),
                               (2, nc.gpsimd), (3, nc.gpsimd)):
                    eng.dma_start(
                        out=G[32 * g:32 * g + 32, :],
                        in_=P[4 * 0 + g:4 * 0 + g + 1, :].rearrange(
                            "p (s f) -> p s f", s=SLOTS),
                    )
                return G

            g_bufs = {0: g_fetch0(), 1: g_fetch(1)}
            h_prev = None
            for i in range(NGT):
                h_cur = l1_front(i, g_bufs.pop(i))
                if i + 2 < NGT:
                    g_bufs[i + 2] = g_fetch(i + 2)
                if i >= 1:
                    l2_back(i - 1, h_prev)
                h_prev = h_cur
            l2_back(NGT - 1, h_prev)

    nc.compile()
    return nc


def _host_consts(pos_logw_x, pos_phi_x, pos_logw_y, pos_phi_y,
                 sac_log_thr, sac_invT, W1, b1, W2, b2, use_b2):
    S_np = np.zeros((128, 128), np.float32)
    for p in range(1, 128):
        if p % CPB != 0:
            S_np[p - 1, p] = 1.0

    t = np.arange(CH, dtype=np.float64) + 1.0
    APOW = np.concatenate([ALPHA_F ** t, ALPHA_S ** t]).astype(np.float32)
    APOW = np.broadcast_to(APOW[None, :], (128, 2 * CH)).copy()

    w_x = np.exp(pos_logw_x.astype(np.float64))
    w_y = np.exp(pos_logw_y.astype(np.float64))
    scal = np.zeros(16, np.float64)
    scal[0:2] = 2.0 * math.pi * w_x * DT   # applied to x/dt
    scal[2:4] = 2.0 * math.pi * w_y * DT
    scal[4:6] = pos_phi_x.astype(np.float64)
    scal[6:8] = pos_phi_y.astype(np.float64)
    scal[8] = float(sac_invT)
    scal[9] = -float(sac_invT) * math.exp(float(sac_log_thr))
    scal[11:13] = pos_phi_x.astype(np.float64) + math.pi / 2.0
    scal[13:15] = pos_phi_y.astype(np.float64) + math.pi / 2.0
    scal[15] = 1e-6
    SCAL = np.broadcast_to(scal.astype(np.float32)[None, :], (128, 16)).copy()
    SCAL[:, 10] = (np.arange(128) % CPB != 0).astype(np.float32)

    W1b = np.zeros((128, 128), np.float32)
    for g in range(4):
        W1b[32 * g:32 * g + 20, :] = W1
        W1b[32 * g + 20, :] = b1
    np_l2 = _np_dt(L2_DT)
    consts = {
        "Smat": S_np, "APOW": APOW, "SCAL": SCAL,
        "W1b": W1b.astype(_np_dt(L1_DT)),
        "W2": np.asarray(W2, np.float32).astype(np_l2),
    }
    if use_b2:
        consts["ones1"] = np.ones((1, 128), np.float32).astype(np_l2)
        consts["b2rep"] = np.tile(
            np.asarray(b2, np.float32), 4)[None, :].astype(np_l2)
    return consts


def kernel(gaze_xy, pos_logw_x, pos_phi_x, pos_logw_y, pos_phi_y,
           sac_log_thr, sac_invT, W1, b1, W2, b2, _trace=False, _tmpdir=None):
    use_b2 = bool(np.any(np.asarray(b2, np.float32)))
    w_x = np.exp(np.asarray(pos_logw_x, np.float64))
    w_y = np.exp(np.asarray(pos_logw_y, np.float64))
    fcoef = tuple(float(2.0 * math.pi * w * DT)
                  for w in list(w_x) + list(w_y))
    fphi = tuple(float(p) for p in
                 list(np.asarray(pos_phi_x, np.float64)) +
                 list(np.asarray(pos_phi_y, np.float64)))
    inv_t = float(sac_invT)
    nthr = -inv_t * math.exp(float(sac_log_thr))
    key = ("nc", use_b2, fcoef, fphi, inv_t, nthr)
    if key not in _cache:
        _cache[key] = _build_nc(use_b2, fcoef, fphi, inv_t, nthr)
    nc = _cache[key]

    consts = _host_consts(pos_logw_x, pos_phi_x, pos_logw_y, pos_phi_y,
                          sac_log_thr, sac_invT, W1, b1, W2, b2, use_b2)
    gaze_xy = np.asarray(gaze_xy, np.float32)
    in_maps = []
    for i in range(N_CORES):
        m = dict(consts)
        m["gaze"] = np.ascontiguousarray(gaze_xy[i * BL:(i + 1) * BL])
        in_maps.append(m)

    res = run_bass_kernel_spmd(nc, in_maps, list(range(N_CORES)),
                               trace=_trace, tmpdir=_tmpdir)
    out = np.concatenate([res.results[i]["out"] for i in range(N_CORES)],
                         0).astype(np.float32)
    if _trace:
        _cache["last_result"] = res
    return out


# revision 17
# speedup vs baseline: 1.0079x; 1.0079x over previous
"""Trainium2 Bass kernel for nn_MinimalGazeEncoder.

Data-parallel over batch: 8 cores x 8 batch elements each.

Per-core layout: partition p = b*16 + c over 128 chunks of 512 timesteps
(b in [0,8), c in [0,16)).  P[128, 21*512] (bf16) holds one [128, 512]
plane per feature channel in reference order (slot 20 = ones for the b1
row); SCR[128, 15*512] (f32) holds full-precision intermediates, so every
feature is computed in f32 and rounded to bf16 exactly once.

Time-shift chunk boundaries (causal diff) and the EMA chunk carries use a
small constant shift matrix on the PE (shift by one chunk within each
batch element); matmul rhs reads the boundary columns via strided views
and the carry consumers read PSUM directly (no staging copies).  The EMA
is a hardware prefix scan (tensor_tensor_scan) plus a rank-1 alpha-powers
carry fixup (alpha^512 underflows so carries never chain).

Matmuls: G-tiles [128, 512] (bf16) gather the 21 feature rows of 4 chunks
onto partitions 32g+s with one SWDGE DMA per tile; layer 1 runs as 4
concurrent row-tiled K=21 bf16 matmuls (W1|b1 replicated at partitions
0/32/64/96).  Layer 2 is flipped (lhsT = strided h1 column views, rhs =
W2): matmul l uses h1 columns {l, 16+l, ..., 2032+l} so the output lands
as [t_block, (l, d)] with t = 16*t_block + l -- each PSUM partition then
maps to a CONTIGUOUS 16-timestep x 128-d run of the HBM output, giving
8 KB DMA descriptors instead of 512 B ones.  The b2 PSUM pre-fill matmuls
are skipped when b2 == 0 (setup_inputs produces zeros); a fill path is
kept for nonzero b2.  The per-tile loop is software-pipelined (L1+gelu1
of tile i runs before L2+gelu2 of tile i-1) so the ACT engine never waits
on the PE round trip with single-buffered PSUM.
"""

import math

import numpy as np
import ml_dtypes

import concourse.bacc as bacc
import concourse.tile as tile
import concourse.mybir as mybir
from concourse.bass_utils import run_bass_kernel_spmd

F32 = mybir.dt.float32
F32R = mybir.dt.float32r
BF16 = mybir.dt.bfloat16
AF = mybir.ActivationFunctionType
ALU = mybir.AluOpType

B, T, D_OUT = 64, 8192, 128
KPOS = 2
DT = 1.0 / 240.0
N_CORES = 8
BL = B // N_CORES          # 8 batch elements per core
CH = 512                   # timesteps per chunk
CPB = T // CH              # 16 chunks per batch element
NP = BL * CPB              # 128 chunks = partitions
GT = 4                     # chunks per G-tile
NGT = NP // GT             # 32 G-tiles per core
SLOTS = 32                 # feature-slot stride in P
NSLOT = 21                 # feature slots (20 features + ones row)

L2_DT = BF16               # layer-2 lhsT/rhs dtype
L1_DT = BF16               # layer-1 lhsT/rhs dtype (G tiles, W1b)

ALPHA_F, ALPHA_S = 0.8, 0.95

# P slot indices (bf16 feature planes).  Order differs from the
# reference feature order (W1b rows are permuted to match on host):
# the early-computed planes (fourier sins, ones) sit contiguously at
# 0..9 so the first G-tile's gather can start before the v/a/EMA chain
# finishes.
S_FX = 0         # 0..3  sin(x,k0) sin(x,k1) cos(x,k0) cos(x,k1)
S_FY = 4         # 4..7
S_ONES = 8
S_VX, S_VY, S_SPD, S_DC, S_DS = 9, 10, 11, 12, 13
S_AX, S_AY, S_APAR, S_APERP = 14, 15, 16, 17
S_GATE, S_QF, S_QS = 18, 19, 20
NSLOT_A = 9      # early G(0) gather: slots [0, 9)

# SCR slot indices (f32 intermediates). XD/YD, VDX/VDY, QF/QS pairs must
# stay adjacent: the carry matmuls read their boundary columns as one
# strided 2-column rhs.  QF/QS reuse the gaze staging slots (stage is
# fully consumed by the XD/YD ops long before the scans run).
C_XD, C_YD, C_VX, C_VY, C_VDX, C_VDY = 0, 1, 2, 3, 4, 5
C_AX, C_AY, C_SPD, C_ISP, C_TA, C_TB, C_TC = 6, 7, 8, 9, 10, 11, 12
C_QF, C_QS, C_TB2, C_TD, C_TE = 13, 14, 15, 16, 17
C_STAGE = 13     # 13..14: raw interleaved gaze staging [128, 1024]

_cache = {}


def _np_dt(dt):
    return np.float32 if dt in (F32, F32R) else ml_dtypes.bfloat16


def _build_nc(use_b2, fcoef, fphi, inv_t, nthr):
    nc = bacc.Bacc("TRN2", target_bir_lowering=False, debug=False,
                   num_devices=N_CORES)

    d_gaze = nc.dram_tensor("gaze", [BL, T, 2], F32, kind="ExternalInput")
    d_W1b = nc.dram_tensor("W1b", [128, 128], L1_DT, kind="ExternalInput")
    d_W2 = nc.dram_tensor("W2", [128, 128], L2_DT, kind="ExternalInput")
    if use_b2:
        d_ones1 = nc.dram_tensor("ones1", [1, 128], L2_DT,
                                 kind="ExternalInput")
        d_b2rep = nc.dram_tensor("b2rep", [1, CH], L2_DT,
                                 kind="ExternalInput")
    d_S = nc.dram_tensor("Smat", [128, 128], F32, kind="ExternalInput")
    d_APOW = nc.dram_tensor("APOW", [128, 2 * CH], F32, kind="ExternalInput")
    d_SCAL = nc.dram_tensor("SCAL", [128, 16], F32, kind="ExternalInput")
    d_out = nc.dram_tensor("out", [BL, T, D_OUT], BF16,
                              kind="ExternalOutput")

    PI = float(np.pi)

    with tile.TileContext(nc) as tc:
        with (
            tc.tile_pool(name="pP", bufs=1) as pP,
            tc.tile_pool(name="pC", bufs=1) as pC,
            tc.tile_pool(name="pG", bufs=5) as pG,
            tc.tile_pool(name="pH", bufs=3) as pH,
            tc.tile_pool(name="pO", bufs=5) as pO,
            tc.tile_pool(name="ps1", bufs=1, space="PSUM") as ps1,
            tc.tile_pool(name="ps2", bufs=2, space="PSUM") as ps2,
        ):
            P = pP.tile([128, SLOTS * CH], BF16)
            SCR = pP.tile([128, 18 * CH], F32)

            def sl(i, n=1):
                return P[:, i * CH:(i + n) * CH]

            def sc(i, n=1):
                return SCR[:, i * CH:(i + n) * CH]

            # gaze staging DMA first: everything in phase A waits on it,
            # so it must head the sync queue.
            stage = sc(C_STAGE, 2)
            gz = d_gaze[:].rearrange("b t two -> b (t two)") \
                          .rearrange("b (c f) -> (b c) f", f=2 * CH)
            nc.sync.dma_start(out=stage[:, 0:CH], in_=gz[:, 0:CH])
            nc.scalar.dma_start(out=stage[:, CH:2 * CH], in_=gz[:, CH:2 * CH])

            # constants / weights on the (idle at startup) ACT queue, in
            # order of first use; the big APOW plane goes last.
            t_SCAL = pC.tile([128, 16], F32, tag="SCAL")
            nc.scalar.dma_start(out=t_SCAL[:], in_=d_SCAL[:])
            t_S = pC.tile([128, 128], F32, tag="Smat")
            nc.scalar.dma_start(out=t_S[:], in_=d_S[:])

            # Preload the trig ACT table (one set is resident at a time;
            # the fourier sins run first) with a dummy [128,1] op while
            # the gaze staging DMA is in flight.
            t_dm = pC.tile([128, 1], F32, tag="tdm")
            nc.gpsimd.memset(t_dm[:], 1.0)
            t_dmo = pC.tile([128, 1], F32, tag="tdmo")
            nc.scalar.activation(t_dmo[:], t_dm[:], AF.Sin)

            t_W1b = pC.tile([128, 128], L1_DT, tag="W1b")
            nc.scalar.dma_start(out=t_W1b[:], in_=d_W1b[:])
            t_W2 = pC.tile([128, 128], L2_DT, tag="W2")
            nc.scalar.dma_start(out=t_W2[:], in_=d_W2[:])
            t_APOW = pC.tile([128, 2 * CH], F32, tag="APOW")
            nc.scalar.dma_start(out=t_APOW[:], in_=d_APOW[:])
            if use_b2:
                t_ones1 = pC.tile([1, 128], L2_DT, tag="ones1")
                nc.scalar.dma_start(out=t_ones1[:], in_=d_ones1[:])
                t_b2rep = pC.tile([1, CH], L2_DT, tag="b2rep")
                nc.scalar.dma_start(out=t_b2rep[:], in_=d_b2rep[:])

            # EMA scan multiplier planes: constant per half, memset on the
            # (idle) Pool engine instead of a 512 KB DMA.
            t_ALPH = pC.tile([128, 2 * CH], F32, tag="ALPH")
            nc.gpsimd.memset(t_ALPH[:, 0:CH], ALPHA_F)
            nc.gpsimd.memset(t_ALPH[:, CH:2 * CH], ALPHA_S)
            nc.gpsimd.memset(sl(S_ONES), 1.0)

            # ---- phase A: feature planes ----
            # Scheduled for DVE/ACT overlap: the v/speed chain runs on DVE
            # while ACT does the cross-engine hops (VD copy, sqrt, +eps,
            # sigmoid, scan-input scales) with tables preloaded; the 8
            # fourier sins and the bf16 casts fill the ACT tail while DVE
            # finishes scans+fixups.
            xs = stage.rearrange("p (t two) -> p two t", two=2)
            x_raw, y_raw = xs[:, 0, :], xs[:, 1, :]

            nc.vector.tensor_scalar_mul(sc(C_XD), x_raw, 1.0 / DT)
            nc.vector.tensor_scalar_mul(sc(C_YD), y_raw, 1.0 / DT)

            def last_cols(base):
                # [128, 2] strided view of the boundary columns of the
                # adjacent planes (base, base+1)
                return SCR[:, base * CH:(base + 2) * CH].rearrange(
                    "p (s f) -> p f s", s=2)[:, CH - 1, :]

            # chunk-boundary carries for v
            psA = ps1.tile([128, 2048], F32, tag="ps1")
            nc.tensor.matmul(psA[:, 0:2], t_S[:], last_cols(C_XD),
                             start=True, stop=True)
            for s_v, s_c, col in ((C_VX, C_XD, 0), (C_VY, C_YD, 1)):
                nc.vector.tensor_tensor(
                    sc(s_v)[:, 1:], sc(s_c)[:, 1:], sc(s_c)[:, :-1],
                    ALU.subtract)
                nc.vector.tensor_tensor(
                    sc(s_v)[:, 0:1], sc(s_c)[:, 0:1], psA[:, col:col + 1],
                    ALU.subtract)

            # first chunk of each batch element: v[0] = 0 (prepended frame)
            v0 = SCR[:, C_VX * CH:(C_VX + 2) * CH].rearrange(
                "p (s f) -> p f s", s=2)[:, 0, :]
            nc.vector.tensor_scalar_mul(v0, v0, t_SCAL[:, 10:11])

            # v/dt on ACT (frees DVE for the speed chain)
            nc.scalar.mul(sc(C_VDX, 2), sc(C_VX, 2), 1.0 / DT)

            # fourier sins early on ACT (trig table already resident, and
            # they only need XD/YD): sin(x*s + phi) via fused scale/bias;
            # cos as sin(arg + pi/2) with host-precomputed phases.  Range
            # reduction is skipped: any residual error sits on unit-scale
            # features contributing ~1e-8 of the output L2 norm.
            for ax_i, (s_base, s_src) in enumerate(
                    ((S_FX, C_XD), (S_FY, C_YD))):
                for k in range(KPOS):
                    wc = 2 * ax_i + k
                    nc.scalar.activation(
                        sl(s_base + k), sc(s_src), AF.Sin,
                        bias=t_SCAL[:, 4 + wc:5 + wc], scale=fcoef[wc])
                    nc.scalar.activation(
                        sl(s_base + KPOS + k), sc(s_src), AF.Sin,
                        bias=t_SCAL[:, 11 + wc:12 + wc], scale=fcoef[wc])

            # speed chain on DVE while ACT copies VD
            nc.vector.tensor_tensor(sc(C_TA, 2), sc(C_VX, 2), sc(C_VX, 2),
                                    ALU.mult)
            nc.vector.tensor_tensor(sc(C_TA), sc(C_TA), sc(C_TB), ALU.add)
            nc.scalar.activation(sc(C_SPD), sc(C_TA), AF.Sqrt)
            nc.scalar.add(sc(C_TB), sc(C_SPD), t_SCAL[:, 15:16])
            # gate = sigmoid(invT*speed - invT*thr), affine fused into ACT
            nc.scalar.activation(sc(C_TC), sc(C_SPD), AF.Sigmoid,
                                 bias=t_SCAL[:, 9:10], scale=inv_t)
            # scan inputs (1-alpha)*gate on ACT
            nc.scalar.mul(sc(C_TA), sc(C_TC), 1.0 - ALPHA_F)
            nc.scalar.mul(sc(C_TB2), sc(C_TC), 1.0 - ALPHA_S)

            # a = diff(v/dt) with carries
            psB = ps1.tile([128, 2048], F32, tag="ps1")
            nc.tensor.matmul(psB[:, 0:2], t_S[:], last_cols(C_VDX),
                             start=True, stop=True)
            for s_a, s_c, col in ((C_AX, C_VDX, 0), (C_AY, C_VDY, 1)):
                nc.vector.tensor_tensor(
                    sc(s_a)[:, 1:], sc(s_c)[:, 1:], sc(s_c)[:, :-1],
                    ALU.subtract)
                nc.vector.tensor_tensor(
                    sc(s_a)[:, 0:1], sc(s_c)[:, 0:1], psB[:, col:col + 1],
                    ALU.subtract)

            # 1/(speed+eps), direction, a_par, a_perp on DVE
            nc.vector.reciprocal_approx_accurate(sc(C_ISP), sc(C_TB),
                                                 sc(C_TD))
            nc.vector.tensor_tensor(sl(S_DC), sc(C_VX), sc(C_ISP), ALU.mult)
            nc.vector.tensor_tensor(sl(S_DS), sc(C_VY), sc(C_ISP), ALU.mult)
            nc.vector.tensor_tensor(sc(C_TD, 2), sc(C_VX, 2), sc(C_AX, 2),
                                    ALU.mult)
            nc.vector.tensor_tensor(sc(C_TD), sc(C_TD), sc(C_TE), ALU.add)
            nc.vector.tensor_tensor(sl(S_APAR), sc(C_TD), sc(C_ISP),
                                    ALU.mult)
            nc.vector.tensor_tensor(sc(C_TD), sc(C_VX), sc(C_AY), ALU.mult)
            nc.vector.tensor_tensor(sc(C_TE), sc(C_VY), sc(C_AX), ALU.mult)
            nc.vector.tensor_tensor(sc(C_TD), sc(C_TD), sc(C_TE),
                                    ALU.subtract)
            nc.vector.tensor_tensor(sl(S_APERP), sc(C_TD), sc(C_ISP),
                                    ALU.mult)

            # EMA scans (within-chunk) + carry fixup
            nc.vector.tensor_tensor_scan(
                sc(C_QF), t_ALPH[:, 0:CH], sc(C_TA), 0.0, ALU.mult, ALU.add)
            nc.vector.tensor_tensor_scan(
                sc(C_QS), t_ALPH[:, CH:2 * CH], sc(C_TB2), 0.0,
                ALU.mult, ALU.add)
            psC = ps1.tile([128, 2048], F32, tag="ps1")
            nc.tensor.matmul(psC[:, 0:2], t_S[:], last_cols(C_QF),
                             start=True, stop=True)
            nc.vector.scalar_tensor_tensor(
                sl(S_QF), t_APOW[:, 0:CH], psC[:, 0:1], sc(C_QF),
                ALU.mult, ALU.add)
            nc.vector.scalar_tensor_tensor(
                sl(S_QS), t_APOW[:, CH:2 * CH], psC[:, 1:2], sc(C_QS),
                ALU.mult, ALU.add)

            # G(0) first wave: gather the fourier+ones slots while the
            # v/a/EMA chain is still running (sync queue is idle).
            G0 = pG.tile([128, CH], L1_DT, tag="G")
            for g in range(GT):
                nc.sync.dma_start(
                    out=G0[32 * g:32 * g + NSLOT_A, :],
                    in_=P[g:g + 1, 0:NSLOT_A * CH].rearrange(
                        "p (s f) -> p s f", s=NSLOT_A),
                )

            # ACT tail: bf16 casts (Copy runs under any resident table).
            nc.scalar.copy(sl(S_VX, 2), sc(C_VX, 2))
            nc.scalar.copy(sl(S_AX, 2), sc(C_AX, 2))
            nc.scalar.copy(sl(S_SPD), sc(C_SPD))
            nc.scalar.copy(sl(S_GATE), sc(C_TC))

            # ---- phase B: software-pipelined per-G-tile matmul loop ----
            def g_fetch(i, eng=None):
                G = pG.tile([128, CH], L1_DT, tag="G")
                (eng or nc.gpsimd).dma_start(
                    out=G[:],
                    in_=P[4 * i:4 * i + 4, :].rearrange(
                        "p (s f) -> p s f", s=SLOTS),
                )
                return G

            def l1_front(i, G):
                ps_l1 = ps1.tile([128, 2048], F32, tag="ps1")
                for g in range(GT):
                    nc.tensor.matmul(
                        ps_l1[:, CH * g:CH * (g + 1)],
                        t_W1b[32 * g:32 * g + NSLOT, :],
                        G[32 * g:32 * g + NSLOT, :],
                        start=True, stop=True,
                        tile_position=(32 * g, 0),
                    )
                h1 = pH.tile([128, 2048], L2_DT, tag="h1")
                nc.scalar.activation(h1[:], ps_l1[:], AF.Gelu)
                return h1

            def l2_back(i, h1):
                # two 2-bank PSUM halves (pool bufs=2): L2 of tile i+1 can
                # start on half a while gelu2(i) is still reading half b,
                # breaking the serial gelu2 -> L2 -> gelu2 chain.
                h1v = h1.rearrange("p (c l) -> p l c", l=16)
                o_t = pO.tile([128, 2048], BF16, tag="o")
                b = (4 * i) // CPB
                c0 = (4 * i) % CPB
                dst = d_out[b, c0 * CH:(c0 + 4) * CH, :].rearrange(
                    "(p l) d -> p (l d)", p=128)
                for h in range(2):
                    ps_l2 = ps2.tile([128, 1024], F32, tag="ps2")
                    if use_b2:
                        for jj in range(2):
                            nc.tensor.matmul(
                                ps_l2[:, CH * jj:CH * (jj + 1)],
                                t_ones1[:], t_b2rep[:],
                                start=True, stop=False, skip_group_check=True)
                        for l in range(8):
                            nc.tensor.matmul(
                                ps_l2[:, 128 * l:128 * (l + 1)],
                                h1v[:, 8 * h + l, :], t_W2[:],
                                start=False, stop=True, skip_group_check=True)
                    else:
                        for l in range(8):
                            nc.tensor.matmul(
                                ps_l2[:, 128 * l:128 * (l + 1)],
                                h1v[:, 8 * h + l, :], t_W2[:],
                                start=True, stop=True)
                    nc.scalar.activation(
                        o_t[:, 1024 * h:1024 * (h + 1)], ps_l2[:], AF.Gelu)
                    if i >= NGT - 2:
                        nc.gpsimd.dma_start(
                            out=dst[:, 1024 * h:1024 * (h + 1)],
                            in_=o_t[:, 1024 * h:1024 * (h + 1)])
                if i < NGT - 2:
                    nc.gpsimd.dma_start(out=dst, in_=o_t[:])

            def g_fetch0_late(G):
                # second wave: the chain-feature slots, per chunk group
                for g, eng in ((0, nc.sync), (1, nc.sync),
                               (2, nc.gpsimd), (3, nc.gpsimd)):
                    eng.dma_start(
                        out=G[32 * g + NSLOT_A:32 * g + NSLOT, :],
                        in_=P[g:g + 1, NSLOT_A * CH:NSLOT * CH].rearrange(
                            "p (s f) -> p s f", s=NSLOT - NSLOT_A),
                    )

            g_fetch0_late(G0)
            g_bufs = {0: G0, 1: g_fetch(1)}
            h_prev = None
            for i in range(NGT):
                h_cur = l1_front(i, g_bufs.pop(i))
                if i + 2 < NGT:
                    g_bufs[i + 2] = g_fetch(i + 2)
                if i >= 1:
                    l2_back(i - 1, h_prev)
                h_prev = h_cur
            l2_back(NGT - 1, h_prev)

    nc.compile()
    return nc


def _host_consts(pos_logw_x, pos_phi_x, pos_logw_y, pos_phi_y,
                 sac_log_thr, sac_invT, W1, b1, W2, b2, use_b2):
    S_np = np.zeros((128, 128), np.float32)
    for p in range(1, 128):
        if p % CPB != 0:
            S_np[p - 1, p] = 1.0

    t = np.arange(CH, dtype=np.float64) + 1.0
    APOW = np.concatenate([ALPHA_F ** t, ALPHA_S ** t]).astype(np.float32)
    APOW = np.broadcast_to(APOW[None, :], (128, 2 * CH)).copy()

    w_x = np.exp(pos_logw_x.astype(np.float64))
    w_y = np.exp(pos_logw_y.astype(np.float64))
    scal = np.zeros(16, np.float64)
    scal[0:2] = 2.0 * math.pi * w_x * DT   # applied to x/dt
    scal[2:4] = 2.0 * math.pi * w_y * DT
    scal[4:6] = pos_phi_x.astype(np.float64)
    scal[6:8] = pos_phi_y.astype(np.float64)
    scal[8] = float(sac_invT)
    scal[9] = -float(sac_invT) * math.exp(float(sac_log_thr))
    scal[11:13] = pos_phi_x.astype(np.float64) + math.pi / 2.0
    scal[13:15] = pos_phi_y.astype(np.float64) + math.pi / 2.0
    scal[15] = 1e-6
    SCAL = np.broadcast_to(scal.astype(np.float32)[None, :], (128, 16)).copy()
    SCAL[:, 10] = (np.arange(128) % CPB != 0).astype(np.float32)

    # slot s -> W1 feature row (None = the b1/ones row), matching the
    # kernel's P slot order
    slot_feat = list(range(8)) + [None] + list(range(8, 20))
    W1b = np.zeros((128, 128), np.float32)
    for g in range(4):
        for s, f in enumerate(slot_feat):
            W1b[32 * g + s, :] = b1 if f is None else W1[f]
    np_l2 = _np_dt(L2_DT)
    consts = {
        "Smat": S_np, "APOW": APOW, "SCAL": SCAL,
        "W1b": W1b.astype(_np_dt(L1_DT)),
        "W2": np.asarray(W2, np.float32).astype(np_l2),
    }
    if use_b2:
        consts["ones1"] = np.ones((1, 128), np.float32).astype(np_l2)
        consts["b2rep"] = np.tile(
            np.asarray(b2, np.float32), 4)[None, :].astype(np_l2)
    return consts


def kernel(gaze_xy, pos_logw_x, pos_phi_x, pos_logw_y, pos_phi_y,
           sac_log_thr, sac_invT, W1, b1, W2, b2, _trace=False, _tmpdir=None):
    use_b2 = bool(np.any(np.asarray(b2, np.float32)))
    w_x = np.exp(np.asarray(pos_logw_x, np.float64))
    w_y = np.exp(np.asarray(pos_logw_y, np.float64))
    fcoef = tuple(float(2.0 * math.pi * w * DT)
                  for w in list(w_x) + list(w_y))
    fphi = tuple(float(p) for p in
                 list(np.asarray(pos_phi_x, np.float64)) +
                 list(np.asarray(pos_phi_y, np.float64)))
    inv_t = float(sac_invT)
    nthr = -inv_t * math.exp(float(sac_log_thr))
    key = ("nc", use_b2, fcoef, fphi, inv_t, nthr)
    if key not in _cache:
        _cache[key] = _build_nc(use_b2, fcoef, fphi, inv_t, nthr)
    nc = _cache[key]

    consts = _host_consts(pos_logw_x, pos_phi_x, pos_logw_y, pos_phi_y,
                          sac_log_thr, sac_invT, W1, b1, W2, b2, use_b2)
    gaze_xy = np.asarray(gaze_xy, np.float32)
    in_maps = []
    for i in range(N_CORES):
        m = dict(consts)
        m["gaze"] = np.ascontiguousarray(gaze_xy[i * BL:(i + 1) * BL])
        in_maps.append(m)

    res = run_bass_kernel_spmd(nc, in_maps, list(range(N_CORES)),
                               trace=_trace, tmpdir=_tmpdir)
    out = np.concatenate([res.results[i]["out"] for i in range(N_CORES)],
                         0).astype(np.float32)
    if _trace:
        _cache["last_result"] = res
    return out
